# revision 19
# baseline (speedup 1.0000x reference)
"""MoNet (GMM graph conv) on Trainium2 — 8-core SPMD Bass/Tile kernel.

Sharding: dst-node slices per core (edge-parallel within core), with node
relabeling into per-core "slot space". Per core, uniform SPMD program:

 - window = 32 dst slots, 5 edge-tiles of 128 slots (3 "even-class" +
   2 "odd-class"); class = src-node table-row parity. Two stride-2 table
   views keep dma_gather's int16 indices in range (26624 rows each).
 - per layer: dma_gather h rows (bf16 256B rows: 64 feats + ones col) ->
   per-tile PE matmul (lhsT = gathered [128,65], rhs = S3' = host 0/1
   one-hot x on-device gauss, built by DVE) accumulating u^T [65,96] per
   window in PSUM -> dense fc matmuls (f32) -> BN via ones-matmul stats +
   AllReduce -> relu (+residual) -> bf16 staging -> AllGather into table.
"""
import sys, os
import numpy as np

if "/opt/trn_rl_repo" not in sys.path:
    sys.path.insert(0, "/opt/trn_rl_repo")

import ml_dtypes
from concourse import bass, bacc, mybir, tile
from concourse import bass_utils
from concourse.masks import make_identity

AluOp = mybir.AluOpType
Act = mybir.ActivationFunctionType
F32 = mybir.dt.float32
BF16 = mybir.dt.bfloat16
I16 = mybir.dt.int16
U16 = mybir.dt.uint16

NCORES = 8
EPS = 1e-5
T_EV, T_OD = 3, 2
TPW = T_EV + T_OD

GEOM_REAL = dict(n=50000, e=800000, in_dim=128, hid=64, k=3, pdim=2,
                 ncls=16, nhl=3, W=208, wpchunk=8)


def derive(geom):
    g = dict(geom)
    g["npc"] = g["W"] * 32                # dst slots per core
    g["NG"] = g["npc"] // 128             # 128-slot groups per core
    g["TPC"] = g["W"] * TPW               # edge tiles per core
    g["NCH"] = g["W"] // g["wpchunk"]     # chunks per layer
    g["TCH"] = g["wpchunk"] * TPW         # tiles per chunk
    g["n_rows"] = NCORES * g["npc"]       # table rows
    g["EVN"] = g["wpchunk"] * T_EV * 128  # idxs per even gather call
    g["ODN"] = g["wpchunk"] * T_OD * 128
    assert g["n_rows"] // 2 <= 32767
    return g


# ---------------------------------------------------------------------------
# host preprocessing (pure integer/index manipulation)
# ---------------------------------------------------------------------------

def preprocess(edge_index, geom):
    g = derive(geom)
    n, W, npc = g["n"], g["W"], g["npc"]
    row = np.asarray(edge_index[0], np.int64)
    col = np.asarray(edge_index[1], np.int64)
    deg_r = np.bincount(row, minlength=n).astype(np.int64)
    deg_c = np.bincount(col, minlength=n).astype(np.int64)

    # 1) nodes -> cores (snake deal by in-degree for balanced edge counts)
    order = np.argsort(-deg_c, kind="stable")
    core_of = np.empty(n, np.int64)
    blk = np.arange(n) // NCORES
    pos = np.arange(n) % NCORES
    snake = np.where(blk % 2 == 0, pos, NCORES - 1 - pos)
    core_of[order] = snake

    # 2) class A (even rows) = per-core top half by out-degree
    is_a = np.zeros(n, bool)
    for c in range(NCORES):
        nds = np.flatnonzero(core_of == c)
        half = min((len(nds) + 1) // 2, W * 16)
        topa = nds[np.argsort(-deg_r[nds], kind="stable")][:half]
        is_a[topa] = True

    src_a = is_a[row]
    in_ev = np.bincount(col[src_a], minlength=n).astype(np.int64)
    in_od = np.bincount(col[~src_a], minlength=n).astype(np.int64)

    # 3) per-core window packing (first-fit decreasing)
    cap_ev, cap_od = T_EV * 128, T_OD * 128
    slot_of = np.full(n, -1, np.int64)
    for c in range(NCORES):
        nds = np.flatnonzero(core_of == c)
        nds = nds[np.argsort(-(in_ev[nds] + in_od[nds]), kind="stable")]
        wev = np.zeros(W, np.int64); wod = np.zeros(W, np.int64)
        wna = np.zeros(W, np.int64); wnb = np.zeros(W, np.int64)
        for nd in nds:
            a = bool(is_a[nd])
            for w in range(W):
                if a and wna[w] >= 16: continue
                if (not a) and wnb[w] >= 16: continue
                if wev[w] + in_ev[nd] > cap_ev: continue
                if wod[w] + in_od[nd] > cap_od: continue
                if a:
                    j = 2 * wna[w]; wna[w] += 1
                else:
                    j = 2 * wnb[w] + 1; wnb[w] += 1
                wev[w] += in_ev[nd]; wod[w] += in_od[nd]
                slot_of[nd] = c * npc + w * 32 + j
                break
            else:
                raise RuntimeError(f"window packing failed (core {c})")

    assert (slot_of >= 0).all()
    # class A nodes landed on even global rows
    assert (slot_of[is_a] % 2 == 0).all() and (slot_of[~is_a] % 2 == 1).all()

    g.update(core_of=core_of, slot_of=slot_of, deg_r=deg_r, deg_c=deg_c)

    # 4) per-core edge-slot layouts
    NCH, TCH, TPC = g["NCH"], g["TCH"], g["TPC"]
    wpc = g["wpchunk"]
    e_core = core_of[col]
    e_slot = slot_of[col] % npc
    e_w = e_slot // 32
    e_j = e_slot % 32
    e_view = (slot_of[row] // 2).astype(np.int64)   # stride-2 view index

    per_core = []
    for c in range(NCORES):
        idx_ev = np.zeros((NCH, 128, g["EVN"] // 16), np.int16)
        idx_od = np.zeros((NCH, 128, g["ODN"] // 16), np.int16)
        eq = np.zeros((NCH, 128, 32, TCH), ml_dtypes.bfloat16)
        dr = np.zeros((128, TPC), np.float32)
        dc = np.zeros((128, TPC), np.float32)

        sel = np.flatnonzero(e_core == c)
        ew, ej, ecls = e_w[sel], e_j[sel], src_a[sel]
        evi = e_view[sel]
        edr = deg_r[row[sel]].astype(np.float32)
        edc = deg_c[col[sel]].astype(np.float32)
        # order edges by (window, class) once
        okey = ew * 2 + (~ecls).astype(np.int64)
        eorder = np.argsort(okey, kind="stable")
        bnd = np.searchsorted(okey[eorder], np.arange(2 * W + 1))
        for w in range(W):
            ch, wl = divmod(w, wpc)
            for a_cls in (True, False):
                kk = w * 2 + (0 if a_cls else 1)
                eids = eorder[bnd[kk]:bnd[kk + 1]]
                ne = len(eids)
                ntile = T_EV if a_cls else T_OD
                base_tti = 0 if a_cls else T_EV
                assert ne <= ntile * 128
                q = np.arange(ne)
                tti = base_tti + q // 128
                p = q % 128
                tg = wl * TPW + tti                      # tile within chunk
                mcall = (wl * ntile + (q // 128)) * 128 + p
                tgt = idx_ev if a_cls else idx_od
                tgt[ch][mcall % 16, mcall // 16] = evi[eids].astype(np.int16)
                eq[ch][p, ej[eids], tg] = 1.0
                dr[p, ch * TCH + tg] = edr[eids]
                dc[p, ch * TCH + tg] = edc[eids]
        idx_ev = np.tile(idx_ev[:, :16, :], (1, 8, 1))
        idx_od = np.tile(idx_od[:, :16, :], (1, 8, 1))
        per_core.append(dict(
            idx_ev=idx_ev, idx_od=idx_od,
            eq=np.ascontiguousarray(eq.reshape(NCH, 128, 32 * TCH)).view(np.uint16),
            dr=dr, dc=dc))
    g["per_core"] = per_core
    return g


# ---------------------------------------------------------------------------
# device program
# ---------------------------------------------------------------------------

def build(tc, outs, ins, g):
    nc = tc.nc
    W, npc, NG = g["W"], g["npc"], g["NG"]
    TPC, NCH, TCH, wpc = g["TPC"], g["NCH"], g["TCH"], g["wpchunk"]
    HID, KK, NCLS, NHL = g["hid"], g["k"], g["ncls"], g["nhl"]
    n_rows = g["n_rows"]
    nn = g["n"]

    import contextlib
    stack = contextlib.ExitStack()
    sbc = stack.enter_context(tc.tile_pool(name="sbc", bufs=1))
    sb1 = stack.enter_context(tc.tile_pool(name="sb1", bufs=1))
    sb = stack.enter_context(tc.tile_pool(name="sb", bufs=2))
    ps = stack.enter_context(tc.tile_pool(name="ps", bufs=6, space="PSUM"))
    dram = stack.enter_context(tc.tile_pool(name="dram", bufs=1, space="DRAM"))

    # ---- constants / persistent state
    onesrow = sbc.tile([1, 128], F32); nc.vector.memset(onesrow[:], 1.0)
    onescol = sbc.tile([128, 1], F32); nc.vector.memset(onescol[:], 1.0)
    ident = sbc.tile([HID, HID], F32)
    nc.sync.dma_start(out=ident[:], in_=ins["ident"][:])
    stage = sbc.tile([128, NG, 128], BF16)
    nc.vector.memset(stage[:], 0.0)
    nc.vector.memset(stage[:, :, 64:65], 1.0)
    srcs = sbc.tile([128, TPC], F32)
    dsts = sbc.tile([128, TPC], F32)
    gauss = sbc.tile([128, KK, TPC], BF16)

    table = dram.tile([n_rows, 128], BF16)
    stage_d = dram.tile([npc, 128], BF16)
    stats_in = dram.tile([HID, 2], F32)
    stats_out = dram.tile([HID, 2], F32)

    zz = sbc.tile([HID, 2], F32)
    nc.vector.memset(zz[:], 0.0)
    nc.sync.dma_start(out=stats_in[:], in_=zz[:])
    nc.sync.dma_start(out=stats_out[:], in_=zz[:])
    # init whole table from the zeroed stage tile (covers pad rows too)
    for c in range(NCORES):
        nc.sync.dma_start(
            out=table[c * npc:(c + 1) * npc, :]
                .rearrange("(gp p) c -> p gp c", p=128),
            in_=stage[:])

    tbl_ev = table[:].rearrange("(m two) c -> m (two c)", two=2)[:, 0:128]
    tbl_od = table[:].rearrange("(m two) c -> m (two c)", two=2)[:, 128:256]

    # ---- prologue: pseudo coords
    with tc.tile_pool(name="pro", bufs=1) as pro:
        drt = pro.tile([128, TPC], F32)
        nc.sync.dma_start(out=drt[:], in_=ins["dr"][:])
        dct = pro.tile([128, TPC], F32)
        nc.sync.dma_start(out=dct[:], in_=ins["dc"][:])
        t0 = pro.tile([128, TPC], F32)
        for dsrc, dout in ((drt, srcs), (dct, dsts)):
            nc.vector.tensor_scalar(t0[:], dsrc[:], 1.0, None, AluOp.add)
            nc.scalar.sqrt(t0[:], t0[:])
            nc.vector.reciprocal(dout[:], t0[:])

    NO_CC = os.environ.get("MONET_NO_CC", "0") == "1"
    NHID_RUN = int(os.environ.get("MONET_NLAYERS", str(NHL)))

    def push_table(h_flat):
        # h_flat [128, NG*64] f32 -> stage bf16 -> DRAM -> AllGather table
        nc.vector.tensor_copy(
            out=stage[:, :, 0:64],
            in_=h_flat.rearrange("p (g c) -> p g c", c=64))
        nc.sync.dma_start(
            out=stage_d[:].rearrange("(gp p) c -> p gp c", p=128),
            in_=stage[:])
        if NO_CC:
            nc.sync.dma_start(out=table[0:npc, :], in_=stage_d[:])
            return
        nc.gpsimd.collective_compute(
            "AllGather", AluOp.bypass, replica_groups=[list(range(NCORES))],
            ins=[stage_d[:].opt()], outs=[table[:].opt()])

    # ---- embed: h0 = featT.T @ emb_w + emb_b
    h_cur = sb.tile([128, NG * HID], F32, tag="h")
    with tc.tile_pool(name="emb", bufs=2) as emb:
        embw = emb.tile([128, HID], F32, tag="embw")
        nc.sync.dma_start(out=embw[:], in_=ins["emb_w"][:])
        ebrow = emb.tile([1, HID], F32, tag="ebrow")
        nc.sync.dma_start(out=ebrow[:], in_=ins["emb_b"][:])
        for gi in range(NG):
            ft = emb.tile([128, 128], F32, tag="ft")
            nc.sync.dma_start(out=ft[:], in_=ins["featT"][:, gi * 128:(gi + 1) * 128])
            ep = ps.tile([128, HID], F32, tag="ps")
            nc.tensor.matmul(out=ep[:], lhsT=ft[:],
                             rhs=embw[:], start=True, stop=True)
            nc.scalar.copy(out=h_cur[:, gi * HID:(gi + 1) * HID], in_=ep[:])
        ebp = ps.tile([128, HID], F32, tag="ps")
        nc.tensor.matmul(out=ebp[:], lhsT=onesrow[:], rhs=ebrow[:],
                         start=True, stop=True)
        ebrep = emb.tile([128, HID], F32)
        nc.scalar.copy(out=ebrep[:], in_=ebp[:])
        nc.vector.tensor_tensor(
            out=h_cur[:], in0=h_cur[:],
            in1=ebrep[:].rearrange("p (o c) -> p o c", o=1)
                .broadcast_to([128, NG, HID]),
            op=AluOp.add)
    push_table(h_cur[:])

    # ---- layers
    for li in list(range(NHID_RUN)) + [NHL]:
        last = li == NHL
        OUT = NCLS if last else HID

        # scalars row: [w00 w01 w10 w11 b0 b1 | mu k*2+d | isg k*2+d]
        scal_row = sb1.tile([1, 32], F32, tag="scalrow")
        nc.vector.memset(scal_row[:], 0.0)
        if last:
            nc.sync.dma_start(out=scal_row[:, 0:4], in_=ins["pp_w_l"][:])
            nc.sync.dma_start(out=scal_row[:, 4:6], in_=ins["pp_b_l"][:])
            nc.sync.dma_start(out=scal_row[:, 6:6 + 2 * KK], in_=ins["mu_l"][:])
            nc.sync.dma_start(out=scal_row[:, 18:18 + 2 * KK],
                              in_=ins["inv_sigma_l"][:])
        else:
            nc.sync.dma_start(out=scal_row[:, 0:4], in_=ins["pp_w"][li])
            nc.sync.dma_start(out=scal_row[:, 4:6], in_=ins["pp_b"][li])
            nc.sync.dma_start(out=scal_row[:, 6:6 + 2 * KK], in_=ins["mu"][li])
            nc.sync.dma_start(out=scal_row[:, 18:18 + 2 * KK],
                              in_=ins["inv_sigma"][li])
        scp = ps.tile([128, 32], F32, tag="ps")
        nc.tensor.matmul(out=scp[:], lhsT=onesrow[:], rhs=scal_row[:],
                         start=True, stop=True)
        scal = sb1.tile([128, 32], F32, tag="scal")
        nc.scalar.copy(out=scal[:], in_=scp[:])

        def sc(j):
            return scal[:, j:j + 1]

        # gauss[k] = exp(-0.5*(((ps0-mu_k0)*is_k0)^2 + ((ps1-mu_k1)*is_k1)^2))
        ps0 = sb1.tile([128, TPC], F32, tag="ps0")
        ps1 = sb1.tile([128, TPC], F32, tag="ps1")
        ta = sb1.tile([128, TPC], F32, tag="ta")
        tb = sb1.tile([128, TPC], F32, tag="tb")
        for (pst, wA, wB, bB) in ((ps0, 0, 2, 4), (ps1, 1, 3, 5)):
            nc.vector.tensor_scalar(ta[:], srcs[:], sc(wA), None, AluOp.mult)
            nc.vector.tensor_scalar(tb[:], dsts[:], sc(wB), None, AluOp.mult)
            nc.vector.tensor_tensor(out=ta[:], in0=ta[:], in1=tb[:], op=AluOp.add)
            nc.scalar.activation(pst[:], ta[:], Act.Tanh, bias=sc(bB), scale=1.0)
        for k in range(KK):
            nc.vector.tensor_scalar(ta[:], ps0[:], sc(6 + 2 * k), sc(18 + 2 * k),
                                    AluOp.subtract, AluOp.mult)
            nc.vector.tensor_scalar(tb[:], ps1[:], sc(7 + 2 * k), sc(19 + 2 * k),
                                    AluOp.subtract, AluOp.mult)
            nc.scalar.square(ta[:], ta[:])
            nc.scalar.square(tb[:], tb[:])
            nc.vector.tensor_tensor(out=ta[:], in0=ta[:], in1=tb[:], op=AluOp.add)
            nc.scalar.activation(gauss[:, k, :], ta[:], Act.Exp,
                                 bias=0.0, scale=-0.5)

        # dense weights [65, K*OUT]
        fcwb = sb1.tile([65, KK * OUT], F32, tag="fcwb")
        if last:
            nc.sync.dma_start(out=fcwb[0:64, :], in_=ins["fc_w_l"][:])
            nc.sync.dma_start(out=fcwb[64:65, :], in_=ins["fc_b_l"][:])
        else:
            nc.sync.dma_start(out=fcwb[0:64, :], in_=ins["fc_w"][li])
            nc.sync.dma_start(out=fcwb[64:65, :], in_=ins["fc_b"][li])

        agg = sb1.tile([128, NG * OUT], F32, tag="aggsb")

        # ---- edge pipeline
        for ch in range(NCH):
            iev = sb.tile([128, g["EVN"] // 16], I16, tag="iev")
            nc.sync.dma_start(out=iev[:], in_=ins["idx_ev"][ch])
            iod = sb.tile([128, g["ODN"] // 16], I16, tag="iod")
            nc.sync.dma_start(out=iod[:], in_=ins["idx_od"][ch])
            eqt = sb.tile([128, 32 * TCH], U16, tag="eq")
            nc.sync.dma_start(out=eqt[:], in_=ins["eq"][ch])
            hg_lo = sb.tile([128, wpc * T_EV, 128], BF16, tag="hglo")
            hg_hi = sb.tile([128, wpc * T_OD, 128], BF16, tag="hghi")
            if os.environ.get("MONET_NO_GATHER", "0") == "1":
                nc.vector.memset(hg_lo[:], 0.5)
                nc.vector.memset(hg_hi[:], 0.5)
            else:
                nc.gpsimd.dma_gather(
                    out_ap=hg_lo[:], in_ap=tbl_ev, idxs_ap=iev[:],
                    num_idxs=g["EVN"], num_idxs_reg=g["EVN"],
                    elem_size=128, elem_step=256, single_packet=False)
                nc.gpsimd.dma_gather(
                    out_ap=hg_hi[:], in_ap=tbl_od, idxs_ap=iod[:],
                    num_idxs=g["ODN"], num_idxs_reg=g["ODN"],
                    elem_size=128, elem_step=256, single_packet=False)
            s3 = sb.tile([128, KK, 32, TCH], BF16, tag="s3")
            eqv = eqt[:].bitcast(BF16).rearrange("p (j t) -> p j t", t=TCH)
            for k in range(KK):
                nc.vector.tensor_tensor(
                    out=s3[:, k], in0=eqv,
                    in1=gauss[:, k, ch * TCH:(ch + 1) * TCH]
                        .rearrange("p (o t) -> p o t", o=1)
                        .broadcast_to([128, 32, TCH]),
                    op=AluOp.mult)
            for wl in range(wpc):
                win = ps.tile([65, KK * 32], F32, tag="ps")
                for tti in range(TPW):
                    tloc = wl * TPW + tti
                    if tti < T_EV:
                        lhs = hg_lo[:, wl * T_EV + tti, 0:65]
                    else:
                        lhs = hg_hi[:, wl * T_OD + (tti - T_EV), 0:65]
                    nc.tensor.matmul(out=win[:], lhsT=lhs,
                                     rhs=s3[:, :, :, tloc],
                                     start=(tti == 0), stop=(tti == TPW - 1))
                sub = wl % 4
                if sub == 0:
                    ust = sb.tile([65, KK, 4, 32], F32, tag="ust")
                nc.scalar.copy(
                    out=ust[:, :, sub, :],
                    in_=win[:].rearrange("u (k j) -> u k j", j=32))
                if sub == 3:
                    gi = (ch * wpc + wl) // 4
                    ap_ = ps.tile([128, OUT], F32, tag="ps")
                    for k in range(KK):
                        lhsu = ust[:, k].rearrange("u a b -> u (a b)")
                        nc.tensor.matmul(
                            out=ap_[:], lhsT=lhsu,
                            rhs=fcwb[:, k * OUT:(k + 1) * OUT],
                            start=(k == 0), stop=(k == KK - 1))
                    nc.scalar.copy(out=agg[:, gi * OUT:(gi + 1) * OUT], in_=ap_[:])

        # ---- BN stats (sum / sumsq over slots via ones-matmul) + AllReduce
        sq = sb1.tile([128, NG * OUT], F32, tag="sq")
        nc.scalar.square(sq[:], agg[:])
        sump = ps.tile([OUT, 1], F32, tag="ps")
        sqp = ps.tile([OUT, 1], F32, tag="ps")
        for gi in range(NG):
            nc.tensor.matmul(out=sump[:], lhsT=agg[:, gi * OUT:(gi + 1) * OUT],
                             rhs=onescol[:], start=(gi == 0), stop=(gi == NG - 1))
            nc.tensor.matmul(out=sqp[:], lhsT=sq[:, gi * OUT:(gi + 1) * OUT],
                             rhs=onescol[:], start=(gi == 0), stop=(gi == NG - 1))
        stats = sb1.tile([OUT, 2], F32, tag="stats")
        nc.scalar.copy(out=stats[:, 0:1], in_=sump[:])
        nc.scalar.copy(out=stats[:, 1:2], in_=sqp[:])
        nc.sync.dma_start(out=stats_in[0:OUT, :], in_=stats[:])
        if NO_CC:
            nc.sync.dma_start(out=stats_out[0:OUT, :], in_=stats_in[0:OUT, :])
        else:
            nc.gpsimd.collective_compute(
                "AllReduce", AluOp.add, replica_groups=[list(range(NCORES))],
                ins=[stats_in[:].opt()], outs=[stats_out[:].opt()])
        stats_ar = sb1.tile([OUT, 2], F32, tag="statsar")
        nc.sync.dma_start(out=stats_ar[:], in_=stats_out[0:OUT, :])
        trp0 = ps.tile([1, OUT], F32, tag="ps")
        nc.tensor.matmul(out=trp0[:], lhsT=stats_ar[:, 0:1],
                         rhs=ident[0:OUT, 0:OUT], start=True, stop=True)
        trp1 = ps.tile([1, OUT], F32, tag="ps")
        nc.tensor.matmul(out=trp1[:], lhsT=stats_ar[:, 1:2],
                         rhs=ident[0:OUT, 0:OUT], start=True, stop=True)
        mean = sb1.tile([1, OUT], F32, tag="mean")
        nc.vector.tensor_scalar(mean[:], trp0[:], 1.0 / nn, None, AluOp.mult)
        ev2 = sb1.tile([1, OUT], F32, tag="ev2")
        nc.vector.tensor_scalar(ev2[:], trp1[:], 1.0 / nn, None, AluOp.mult)
        m2 = sb1.tile([1, OUT], F32, tag="m2")
        nc.vector.tensor_tensor(out=m2[:], in0=mean[:], in1=mean[:], op=AluOp.mult)
        var = sb1.tile([1, OUT], F32, tag="var")
        nc.vector.tensor_tensor(out=var[:], in0=ev2[:], in1=m2[:], op=AluOp.subtract)
        nc.vector.tensor_scalar(var[:], var[:], EPS, None, AluOp.add)
        std = sb1.tile([1, OUT], F32, tag="std")
        nc.scalar.sqrt(std[:], var[:])
        rstd = sb1.tile([1, OUT], F32, tag="rstd")
        nc.vector.reciprocal(rstd[:], std[:])
        bng = sb1.tile([1, OUT], F32, tag="bng")
        bnb = sb1.tile([1, OUT], F32, tag="bnb")
        if last:
            nc.sync.dma_start(out=bng[:], in_=ins["bn_g_l"][:])
            nc.sync.dma_start(out=bnb[:], in_=ins["bn_b_l"][:])
        else:
            nc.sync.dma_start(out=bng[:], in_=ins["bn_g"][li])
            nc.sync.dma_start(out=bnb[:], in_=ins["bn_b"][li])
        sg = sb1.tile([1, OUT], F32, tag="sg")
        nc.vector.tensor_tensor(out=sg[:], in0=rstd[:], in1=bng[:], op=AluOp.mult)
        c0 = sb1.tile([1, OUT], F32, tag="c0")
        nc.vector.tensor_tensor(out=c0[:], in0=mean[:], in1=sg[:], op=AluOp.mult)
        crow = sb1.tile([1, OUT], F32, tag="crow")
        nc.vector.tensor_tensor(out=crow[:], in0=bnb[:], in1=c0[:], op=AluOp.subtract)
        reps = []
        for rsrc in (sg, crow):
            rp = ps.tile([128, OUT], F32, tag="ps")
            nc.tensor.matmul(out=rp[:], lhsT=onesrow[:], rhs=rsrc[:],
                             start=True, stop=True)
            rt = sb1.tile([128, OUT], F32, tag=f"rep{len(reps)}")
            nc.scalar.copy(out=rt[:], in_=rp[:])
            reps.append(rt)

        def rep_b(rt):
            return rt[:].rearrange("p (o c) -> p o c", o=1).broadcast_to([128, NG, OUT])

        bn = sq  # reuse buffer
        aggv = agg[:].rearrange("p (g c) -> p g c", c=OUT)
        bnv = bn[:].rearrange("p (g c) -> p g c", c=OUT)
        nc.vector.tensor_tensor(out=bnv, in0=aggv, in1=rep_b(reps[0]), op=AluOp.mult)
        nc.vector.tensor_tensor(out=bnv, in0=bnv, in1=rep_b(reps[1]), op=AluOp.add)
        nc.vector.tensor_scalar(bn[:], bn[:], 0.0, None, AluOp.max)

        if last:
            nc.sync.dma_start(out=outs["out"][:], in_=bn[:])
        else:
            h_new = sb.tile([128, NG * HID], F32, tag="h")
            nc.vector.tensor_tensor(out=h_new[:], in0=bn[:], in1=h_cur[:],
                                    op=AluOp.add)
            h_cur = h_new
            push_table(h_cur[:])

    stack.close()


# ---------------------------------------------------------------------------
# top-level entry
# ---------------------------------------------------------------------------

def _make_in_maps(g, weights):
    in_maps = []
    for c in range(NCORES):
        pc = g["per_core"][c]
        m = dict(weights)
        m["featT"] = g["featT"][c]
        m["ident"] = np.eye(g["hid"], dtype=np.float32)
        m["idx_ev"] = pc["idx_ev"]
        m["idx_od"] = pc["idx_od"]
        m["eq"] = pc["eq"]
        m["dr"] = pc["dr"]
        m["dc"] = pc["dc"]
        in_maps.append({k + "_d": v for k, v in m.items()})
    return in_maps


def _weights_dict(inputs, g):
    f32 = lambda x: np.ascontiguousarray(np.asarray(x, np.float32))
    w = dict(
        emb_w=f32(inputs["emb_w"]),                       # [128, 64]
        emb_b=f32(inputs["emb_b"]).reshape(1, -1),
        fc_w=f32(inputs["fc_w"]),                         # [3, 64, 192]
        fc_b=f32(inputs["fc_b"]).reshape(g["nhl"], 1, -1),
        mu=f32(inputs["mu"]).reshape(g["nhl"], 1, -1),
        inv_sigma=f32(inputs["inv_sigma"]).reshape(g["nhl"], 1, -1),
        pp_w=f32(inputs["pp_w"]).reshape(g["nhl"], 1, -1),
        pp_b=f32(inputs["pp_b"]).reshape(g["nhl"], 1, -1),
        bn_g=f32(inputs["bn_g"]).reshape(g["nhl"], 1, -1),
        bn_b=f32(inputs["bn_b"]).reshape(g["nhl"], 1, -1),
        fc_w_l=f32(inputs["fc_w_l"]), fc_b_l=f32(inputs["fc_b_l"]).reshape(1, -1),
        mu_l=f32(inputs["mu_l"]).reshape(1, -1),
        inv_sigma_l=f32(inputs["inv_sigma_l"]).reshape(1, -1),
        pp_w_l=f32(inputs["pp_w_l"]).reshape(1, -1),
        pp_b_l=f32(inputs["pp_b_l"]).reshape(1, -1),
        bn_g_l=f32(inputs["bn_g_l"]).reshape(1, -1),
        bn_b_l=f32(inputs["bn_b_l"]).reshape(1, -1),
    )
    return w


def _build_featT(inputs, g):
    feat = np.asarray(inputs["feature"], np.float32)
    featT = []
    for c in range(NCORES):
        arr = np.zeros((g["in_dim"], g["npc"]), np.float32)
        nds = np.flatnonzero(g["core_of"] == c)
        arr[:, g["slot_of"][nds] % g["npc"]] = feat[nds].T
        featT.append(arr)
    g["featT"] = featT


def run_device(g, weights, trace=False):
    nc = bacc.Bacc("TRN2", target_bir_lowering=False, debug=False,
                   num_devices=NCORES)
    ins_ap, outs_ap = {}, {}
    in_maps = _make_in_maps(g, weights)
    for name, arr in in_maps[0].items():
        t = nc.dram_tensor(name, list(arr.shape), mybir.dt.from_np(arr.dtype),
                           kind="ExternalInput")
        ins_ap[name[:-2]] = t.ap()
    out_t = nc.dram_tensor("out_d", [128, g["NG"] * g["ncls"]], F32,
                           kind="ExternalOutput")
    outs_ap["out"] = out_t.ap()

    with tile.TileContext(nc) as tc:
        build(tc, outs_ap, ins_ap, g)
    nc.compile()

    res = bass_utils.run_bass_kernel_spmd(
        nc, in_maps, core_ids=list(range(NCORES)), trace=trace)
    return res


def assemble_output(g, res):
    out = np.zeros((g["n"], g["ncls"]), np.float32)
    for c in range(NCORES):
        oc = res.results[c]["out_d"].reshape(128, g["NG"], g["ncls"])
        nds = np.flatnonzero(g["core_of"] == c)
        sl = g["slot_of"][nds] % g["npc"]
        out[nds] = oc[sl % 128, sl // 128, :]
    return out


def kernel(**inputs):
    g = preprocess(np.asarray(inputs["edge_index"]), GEOM_REAL)
    _build_featT(inputs, g)
    weights = _weights_dict(inputs, g)
    res = run_device(g, weights, trace=os.environ.get("MONET_TRACE", "0") == "1")
    out = assemble_output(g, res)
    kernel.last_exec_time_ns = getattr(res, "exec_time_ns", None)
    return out


# ---------------------------------------------------------------------------
# numpy reference (dev only; mirrors reference.py)
# ---------------------------------------------------------------------------

def numpy_reference(inputs, n, nhl=3):
    f = {k: np.asarray(v, np.float64 if np.asarray(v).dtype.kind == "f" else None)
         for k, v in inputs.items()}
    row, col = np.asarray(inputs["edge_index"][0]), np.asarray(inputs["edge_index"][1])
    deg_r = np.bincount(row, minlength=n)
    deg_c = np.bincount(col, minlength=n)
    srcs = 1.0 / np.sqrt(deg_r[row] + 1.0)
    dsts = 1.0 / np.sqrt(deg_c[col] + 1.0)
    pseudo = np.stack([srcs, dsts], -1)
    h = f["feature"] @ f["emb_w"] + f["emb_b"]

    def gmm(h, psd, fcw, fcb, mu, isg, bng, bnb, residual):
        kk, out = mu.shape[0], fcw.shape[1] // mu.shape[0]
        hp = (h @ fcw + fcb).reshape(n, kk, out)
        diff = psd[:, None, :] - mu
        gauss = np.exp(-0.5 * np.sum((diff * isg) ** 2, -1))
        msg = np.einsum("ek,ekc->ec", gauss, hp[row])
        agg = np.zeros((n, out))
        np.add.at(agg, col, msg)
        mean = agg.mean(0)
        var = agg.var(0)
        hbn = (agg - mean) / np.sqrt(var + EPS) * bng + bnb
        hnew = np.maximum(hbn, 0.0)
        return h + hnew if residual else hnew

    for i in range(nhl):
        psd = np.tanh(pseudo @ f["pp_w"][i] + f["pp_b"][i])
        h = gmm(h, psd, f["fc_w"][i], f["fc_b"][i], f["mu"][i],
                f["inv_sigma"][i], f["bn_g"][i], f["bn_b"][i], True)
    psd = np.tanh(pseudo @ f["pp_w_l"] + f["pp_b_l"])
    h = gmm(h, psd, f["fc_w_l"], f["fc_b_l"], f["mu_l"], f["inv_sigma_l"],
            f["bn_g_l"], f["bn_b_l"], False)
    return h.astype(np.float32)


# ---------------------------------------------------------------------------
# timed execution (repeated PJRT calls on a single compiled executable)
# ---------------------------------------------------------------------------

def run_device_timed(g, weights, n_iters=5):
    import time
    import jax
    from jax.sharding import Mesh, PartitionSpec
    from jax.experimental.shard_map import shard_map
    from concourse import bass2jax as b2j

    nc = bacc.Bacc("TRN2", target_bir_lowering=False, debug=False,
                   num_devices=NCORES)
    ins_ap = {}
    in_maps = _make_in_maps(g, weights)
    for name, arr in in_maps[0].items():
        t = nc.dram_tensor(name, list(arr.shape), mybir.dt.from_np(arr.dtype),
                           kind="ExternalInput")
        ins_ap[name[:-2]] = t.ap()
    out_t = nc.dram_tensor("out_d", [128, g["NG"] * g["ncls"]], F32,
                           kind="ExternalOutput")
    outs_ap = {"out": out_t.ap()}
    with tile.TileContext(nc) as tc:
        build(tc, outs_ap, ins_ap, g)
    nc.compile()

    b2j.install_neuronx_cc_hook()
    partition_name = (nc.partition_id_tensor.name
                      if nc.partition_id_tensor else None)
    in_names, out_names, out_avals, zero_outs = [], [], [], []
    for alloc in nc.m.functions[0].allocations:
        if not isinstance(alloc, mybir.MemoryLocationSet):
            continue
        name = alloc.memorylocations[0].name
        if alloc.kind == "ExternalInput":
            if name != partition_name:
                in_names.append(name)
        elif alloc.kind == "ExternalOutput":
            dt = mybir.dt.np(alloc.dtype)
            out_avals.append(jax.core.ShapedArray(tuple(alloc.tensor_shape), dt))
            out_names.append(name)
            zero_outs.append(np.zeros(tuple(alloc.tensor_shape), dt))
    n_params = len(in_names)
    n_outs = len(out_names)
    in_names = in_names + out_names
    if partition_name is not None:
        in_names.append(partition_name)
    donate = tuple(range(n_params, n_params + n_outs))

    def _body(*args):
        operands = list(args)
        if partition_name is not None:
            operands.append(b2j.partition_id_tensor())
        outs = b2j._bass_exec_p.bind(
            *operands,
            out_avals=tuple(out_avals),
            in_names=tuple(in_names),
            out_names=tuple(out_names),
            lowering_input_output_aliases=(),
            sim_require_finite=True,
            sim_require_nnan=True,
            nc=nc,
        )
        return tuple(outs)

    devices = jax.devices()[:NCORES]
    mesh = Mesh(np.asarray(devices), ("core",))
    sharded = jax.jit(
        shard_map(_body, mesh=mesh,
                  in_specs=(PartitionSpec("core"),) * (n_params + n_outs),
                  out_specs=(PartitionSpec("core"),) * n_outs,
                  check_rep=False),
        donate_argnums=donate, keep_unused=True)
    per_core = [[np.asarray(m[nm]) for nm in in_names[:n_params]]
                for m in in_maps]
    concat_in = [np.concatenate([per_core[c][i] for c in range(NCORES)], 0)
                 for i in range(n_params)]
    concat_in = [jax.device_put(a) for a in concat_in]

    times = []
    out_arrs = None
    for it in range(n_iters):
        czeros = [np.zeros((NCORES * z.shape[0], *z.shape[1:]), z.dtype)
                  for z in zero_outs]
        t0 = time.perf_counter()
        out_arrs = sharded(*concat_in, *czeros)
        jax.block_until_ready(out_arrs)
        times.append(time.perf_counter() - t0)
    results = [
        {nm: np.asarray(out_arrs[i]).reshape(NCORES, *out_avals[i].shape)[c]
         for i, nm in enumerate(out_names)}
        for c in range(NCORES)
    ]

    class R:
        pass
    r = R()
    r.results = results
    r.exec_time_ns = int(min(times[1:]) * 1e9) if len(times) > 1 else None
    r.all_times = times
    return r


# revision 21
# speedup vs baseline: 1.0668x; 1.0668x over previous
"""MoNet (GMM graph conv) on Trainium2 — 8-core SPMD Bass/Tile kernel.

Sharding: dst-node slices per core (edge-parallel within core), with node
relabeling into per-core "slot space". Per core, uniform SPMD program:

 - window = 32 dst slots, 5 edge-tiles of 128 slots (3 "even-class" +
   2 "odd-class"); class = src-node table-row parity. Two stride-2 table
   views keep dma_gather's int16 indices in range (26624 rows each).
 - per layer: dma_gather h rows (bf16 256B rows: 64 feats + ones col) ->
   per-tile PE matmul (lhsT = gathered [128,65], rhs = S3' = host 0/1
   one-hot x on-device gauss, built by DVE) accumulating u^T [65,96] per
   window in PSUM -> dense fc matmuls (f32) -> BN via ones-matmul stats +
   AllReduce -> relu (+residual) -> bf16 staging -> AllGather into table.
"""
import sys, os
import numpy as np

if "/opt/trn_rl_repo" not in sys.path:
    sys.path.insert(0, "/opt/trn_rl_repo")

import ml_dtypes
from concourse import bass, bacc, mybir, tile
from concourse import bass_utils
from concourse.masks import make_identity

AluOp = mybir.AluOpType
Act = mybir.ActivationFunctionType
F32 = mybir.dt.float32
BF16 = mybir.dt.bfloat16
I16 = mybir.dt.int16
U16 = mybir.dt.uint16

NCORES = 8
EPS = 1e-5
T_EV, T_OD = 3, 2
TPW = T_EV + T_OD

GEOM_REAL = dict(n=50000, e=800000, in_dim=128, hid=64, k=3, pdim=2,
                 ncls=16, nhl=3, W=208, wpchunk=16)


def derive(geom):
    g = dict(geom)
    g["npc"] = g["W"] * 32                # dst slots per core
    g["NG"] = g["npc"] // 128             # 128-slot groups per core
    g["TPC"] = g["W"] * TPW               # edge tiles per core
    g["NCH"] = g["W"] // g["wpchunk"]     # chunks per layer
    g["TCH"] = g["wpchunk"] * TPW         # tiles per chunk
    g["n_rows"] = NCORES * g["npc"]       # table rows
    g["EVN"] = g["wpchunk"] * T_EV * 128  # idxs per even gather call
    g["ODN"] = g["wpchunk"] * T_OD * 128
    assert g["n_rows"] // 2 <= 32767
    return g


# ---------------------------------------------------------------------------
# host preprocessing (pure integer/index manipulation)
# ---------------------------------------------------------------------------

def preprocess(edge_index, geom):
    g = derive(geom)
    n, W, npc = g["n"], g["W"], g["npc"]
    row = np.asarray(edge_index[0], np.int64)
    col = np.asarray(edge_index[1], np.int64)
    deg_r = np.bincount(row, minlength=n).astype(np.int64)
    deg_c = np.bincount(col, minlength=n).astype(np.int64)

    # 1) nodes -> cores (snake deal by in-degree for balanced edge counts)
    order = np.argsort(-deg_c, kind="stable")
    core_of = np.empty(n, np.int64)
    blk = np.arange(n) // NCORES
    pos = np.arange(n) % NCORES
    snake = np.where(blk % 2 == 0, pos, NCORES - 1 - pos)
    core_of[order] = snake

    # 2) class A (even rows) = per-core top half by out-degree
    is_a = np.zeros(n, bool)
    for c in range(NCORES):
        nds = np.flatnonzero(core_of == c)
        half = min((len(nds) + 1) // 2, W * 16)
        topa = nds[np.argsort(-deg_r[nds], kind="stable")][:half]
        is_a[topa] = True

    src_a = is_a[row]
    in_ev = np.bincount(col[src_a], minlength=n).astype(np.int64)
    in_od = np.bincount(col[~src_a], minlength=n).astype(np.int64)

    # 3) per-core window packing (first-fit decreasing)
    cap_ev, cap_od = T_EV * 128, T_OD * 128
    slot_of = np.full(n, -1, np.int64)
    for c in range(NCORES):
        nds = np.flatnonzero(core_of == c)
        nds = nds[np.argsort(-(in_ev[nds] + in_od[nds]), kind="stable")]
        wev = np.zeros(W, np.int64); wod = np.zeros(W, np.int64)
        wna = np.zeros(W, np.int64); wnb = np.zeros(W, np.int64)
        for nd in nds:
            a = bool(is_a[nd])
            for w in range(W):
                if a and wna[w] >= 16: continue
                if (not a) and wnb[w] >= 16: continue
                if wev[w] + in_ev[nd] > cap_ev: continue
                if wod[w] + in_od[nd] > cap_od: continue
                if a:
                    j = 2 * wna[w]; wna[w] += 1
                else:
                    j = 2 * wnb[w] + 1; wnb[w] += 1
                wev[w] += in_ev[nd]; wod[w] += in_od[nd]
                slot_of[nd] = c * npc + w * 32 + j
                break
            else:
                raise RuntimeError(f"window packing failed (core {c})")

    assert (slot_of >= 0).all()
    # class A nodes landed on even global rows
    assert (slot_of[is_a] % 2 == 0).all() and (slot_of[~is_a] % 2 == 1).all()

    g.update(core_of=core_of, slot_of=slot_of, deg_r=deg_r, deg_c=deg_c)

    # 4) per-core edge-slot layouts
    NCH, TCH, TPC = g["NCH"], g["TCH"], g["TPC"]
    wpc = g["wpchunk"]
    e_core = core_of[col]
    e_slot = slot_of[col] % npc
    e_w = e_slot // 32
    e_j = e_slot % 32
    e_view = (slot_of[row] // 2).astype(np.int64)   # stride-2 view index

    per_core = []
    for c in range(NCORES):
        idx_ev = np.zeros((NCH, 128, g["EVN"] // 16), np.int16)
        idx_od = np.zeros((NCH, 128, g["ODN"] // 16), np.int16)
        eq = np.zeros((NCH, 128, 32, TCH), ml_dtypes.bfloat16)
        dr = np.zeros((128, TPC), np.float32)
        dc = np.zeros((128, TPC), np.float32)

        sel = np.flatnonzero(e_core == c)
        ew, ej, ecls = e_w[sel], e_j[sel], src_a[sel]
        evi = e_view[sel]
        edr = deg_r[row[sel]].astype(np.float32)
        edc = deg_c[col[sel]].astype(np.float32)
        # order edges by (window, class) once
        okey = ew * 2 + (~ecls).astype(np.int64)
        eorder = np.argsort(okey, kind="stable")
        bnd = np.searchsorted(okey[eorder], np.arange(2 * W + 1))
        for w in range(W):
            ch, wl = divmod(w, wpc)
            for a_cls in (True, False):
                kk = w * 2 + (0 if a_cls else 1)
                eids = eorder[bnd[kk]:bnd[kk + 1]]
                ne = len(eids)
                ntile = T_EV if a_cls else T_OD
                base_tti = 0 if a_cls else T_EV
                assert ne <= ntile * 128
                q = np.arange(ne)
                tti = base_tti + q // 128
                p = q % 128
                tg = wl * TPW + tti                      # tile within chunk
                mcall = (wl * ntile + (q // 128)) * 128 + p
                tgt = idx_ev if a_cls else idx_od
                tgt[ch][mcall % 16, mcall // 16] = evi[eids].astype(np.int16)
                eq[ch][p, ej[eids], tg] = 1.0
                dr[p, ch * TCH + tg] = edr[eids]
                dc[p, ch * TCH + tg] = edc[eids]
        idx_ev = np.tile(idx_ev[:, :16, :], (1, 8, 1))
        idx_od = np.tile(idx_od[:, :16, :], (1, 8, 1))
        per_core.append(dict(
            idx_ev=idx_ev, idx_od=idx_od,
            eq=np.ascontiguousarray(eq.reshape(NCH, 128, 32 * TCH)).view(np.uint16),
            dr=dr, dc=dc))
    g["per_core"] = per_core
    return g


# ---------------------------------------------------------------------------
# device program
# ---------------------------------------------------------------------------

def build(tc, outs, ins, g):
    nc = tc.nc
    W, npc, NG = g["W"], g["npc"], g["NG"]
    TPC, NCH, TCH, wpc = g["TPC"], g["NCH"], g["TCH"], g["wpchunk"]
    HID, KK, NCLS, NHL = g["hid"], g["k"], g["ncls"], g["nhl"]
    n_rows = g["n_rows"]
    nn = g["n"]

    import contextlib
    stack = contextlib.ExitStack()
    sbc = stack.enter_context(tc.tile_pool(name="sbc", bufs=1))
    sb1 = stack.enter_context(tc.tile_pool(name="sb1", bufs=1))
    sb = stack.enter_context(tc.tile_pool(name="sb", bufs=2))
    ps = stack.enter_context(tc.tile_pool(name="ps", bufs=8, space="PSUM"))
    dram = stack.enter_context(tc.tile_pool(name="dram", bufs=1, space="DRAM"))

    # ---- constants / persistent state
    onesrow = sbc.tile([1, 128], F32); nc.vector.memset(onesrow[:], 1.0)
    onescol = sbc.tile([128, 1], F32); nc.vector.memset(onescol[:], 1.0)
    ident = sbc.tile([HID, HID], F32)
    nc.sync.dma_start(out=ident[:], in_=ins["ident"][:])
    stage = sbc.tile([128, NG, 128], BF16)
    nc.vector.memset(stage[:], 0.0)
    nc.vector.memset(stage[:, :, 64:65], 1.0)
    srcs = sbc.tile([128, TPC], F32)
    dsts = sbc.tile([128, TPC], F32)
    gauss = sbc.tile([128, KK, TPC], BF16)

    table = dram.tile([n_rows, 128], BF16)
    stage_d = dram.tile([npc, 128], BF16)
    stats_in = dram.tile([HID, 2], F32)
    stats_out = dram.tile([HID, 2], F32)

    zz = sbc.tile([HID, 2], F32)
    nc.vector.memset(zz[:], 0.0)
    nc.sync.dma_start(out=stats_in[:], in_=zz[:])
    nc.sync.dma_start(out=stats_out[:], in_=zz[:])
    # init whole table from the zeroed stage tile (covers pad rows too)
    for c in range(NCORES):
        nc.sync.dma_start(
            out=table[c * npc:(c + 1) * npc, :]
                .rearrange("(gp p) c -> p gp c", p=128),
            in_=stage[:])

    tbl_ev = table[:].rearrange("(m two) c -> m (two c)", two=2)[:, 0:128]
    tbl_od = table[:].rearrange("(m two) c -> m (two c)", two=2)[:, 128:256]

    # ---- prologue: pseudo coords
    with tc.tile_pool(name="pro", bufs=1) as pro:
        drt = pro.tile([128, TPC], F32)
        nc.sync.dma_start(out=drt[:], in_=ins["dr"][:])
        dct = pro.tile([128, TPC], F32)
        nc.sync.dma_start(out=dct[:], in_=ins["dc"][:])
        t0 = pro.tile([128, TPC], F32)
        for dsrc, dout in ((drt, srcs), (dct, dsts)):
            nc.vector.tensor_scalar(t0[:], dsrc[:], 1.0, None, AluOp.add)
            nc.scalar.sqrt(t0[:], t0[:])
            nc.vector.reciprocal(dout[:], t0[:])

    NO_CC = os.environ.get("MONET_NO_CC", "0") == "1"
    NHID_RUN = int(os.environ.get("MONET_NLAYERS", str(NHL)))

    def push_table(h_flat):
        # h_flat [128, NG*64] f32 -> stage bf16 -> DRAM -> AllGather table
        nc.vector.tensor_copy(
            out=stage[:, :, 0:64],
            in_=h_flat.rearrange("p (g c) -> p g c", c=64))
        nc.sync.dma_start(
            out=stage_d[:].rearrange("(gp p) c -> p gp c", p=128),
            in_=stage[:])
        if NO_CC:
            nc.sync.dma_start(out=table[0:npc, :], in_=stage_d[:])
            return
        nc.gpsimd.collective_compute(
            "AllGather", AluOp.bypass, replica_groups=[list(range(NCORES))],
            ins=[stage_d[:].opt()], outs=[table[:].opt()])

    # ---- embed: h0 = featT.T @ emb_w + emb_b
    h_cur = sb.tile([128, NG * HID], F32, tag="h")
    with tc.tile_pool(name="emb", bufs=2) as emb:
        embw = emb.tile([128, HID], F32, tag="embw")
        nc.sync.dma_start(out=embw[:], in_=ins["emb_w"][:])
        ebrow = emb.tile([1, HID], F32, tag="ebrow")
        nc.sync.dma_start(out=ebrow[:], in_=ins["emb_b"][:])
        for gi in range(NG):
            ft = emb.tile([128, 128], F32, tag="ft")
            nc.sync.dma_start(out=ft[:], in_=ins["featT"][:, gi * 128:(gi + 1) * 128])
            ep = ps.tile([128, HID], F32, tag="ps")
            nc.tensor.matmul(out=ep[:], lhsT=ft[:],
                             rhs=embw[:], start=True, stop=True)
            nc.scalar.copy(out=h_cur[:, gi * HID:(gi + 1) * HID], in_=ep[:])
        ebp = ps.tile([128, HID], F32, tag="ps")
        nc.tensor.matmul(out=ebp[:], lhsT=onesrow[:], rhs=ebrow[:],
                         start=True, stop=True)
        ebrep = emb.tile([128, HID], F32)
        nc.scalar.copy(out=ebrep[:], in_=ebp[:])
        nc.vector.tensor_tensor(
            out=h_cur[:], in0=h_cur[:],
            in1=ebrep[:].rearrange("p (o c) -> p o c", o=1)
                .broadcast_to([128, NG, HID]),
            op=AluOp.add)
    push_table(h_cur[:])

    # ---- layers
    for li in list(range(NHID_RUN)) + [NHL]:
        last = li == NHL
        OUT = NCLS if last else HID

        # scalars row: [w00 w01 w10 w11 b0 b1 | mu k*2+d | isg k*2+d]
        scal_row = sb1.tile([1, 32], F32, tag="scalrow")
        nc.vector.memset(scal_row[:], 0.0)
        if last:
            nc.sync.dma_start(out=scal_row[:, 0:4], in_=ins["pp_w_l"][:])
            nc.sync.dma_start(out=scal_row[:, 4:6], in_=ins["pp_b_l"][:])
            nc.sync.dma_start(out=scal_row[:, 6:6 + 2 * KK], in_=ins["mu_l"][:])
            nc.sync.dma_start(out=scal_row[:, 18:18 + 2 * KK],
                              in_=ins["inv_sigma_l"][:])
        else:
            nc.sync.dma_start(out=scal_row[:, 0:4], in_=ins["pp_w"][li])
            nc.sync.dma_start(out=scal_row[:, 4:6], in_=ins["pp_b"][li])
            nc.sync.dma_start(out=scal_row[:, 6:6 + 2 * KK], in_=ins["mu"][li])
            nc.sync.dma_start(out=scal_row[:, 18:18 + 2 * KK],
                              in_=ins["inv_sigma"][li])
        scp = ps.tile([128, 32], F32, tag="ps")
        nc.tensor.matmul(out=scp[:], lhsT=onesrow[:], rhs=scal_row[:],
                         start=True, stop=True)
        scal = sb1.tile([128, 32], F32, tag="scal")
        nc.scalar.copy(out=scal[:], in_=scp[:])

        def sc(j):
            return scal[:, j:j + 1]

        # gauss[k] = exp(-0.5*(((ps0-mu_k0)*is_k0)^2 + ((ps1-mu_k1)*is_k1)^2))
        ps0 = sb1.tile([128, TPC], F32, tag="ps0")
        ps1 = sb1.tile([128, TPC], F32, tag="ps1")
        ta = sb1.tile([128, TPC], F32, tag="ta")
        tb = sb1.tile([128, TPC], F32, tag="tb")
        for (pst, wA, wB, bB) in ((ps0, 0, 2, 4), (ps1, 1, 3, 5)):
            nc.vector.tensor_scalar(ta[:], srcs[:], sc(wA), None, AluOp.mult)
            nc.vector.tensor_scalar(tb[:], dsts[:], sc(wB), None, AluOp.mult)
            nc.vector.tensor_tensor(out=ta[:], in0=ta[:], in1=tb[:], op=AluOp.add)
            nc.scalar.activation(pst[:], ta[:], Act.Tanh, bias=sc(bB), scale=1.0)
        for k in range(KK):
            nc.vector.tensor_scalar(ta[:], ps0[:], sc(6 + 2 * k), sc(18 + 2 * k),
                                    AluOp.subtract, AluOp.mult)
            nc.vector.tensor_scalar(tb[:], ps1[:], sc(7 + 2 * k), sc(19 + 2 * k),
                                    AluOp.subtract, AluOp.mult)
            nc.scalar.square(ta[:], ta[:])
            nc.scalar.square(tb[:], tb[:])
            nc.vector.tensor_tensor(out=ta[:], in0=ta[:], in1=tb[:], op=AluOp.add)
            nc.scalar.activation(gauss[:, k, :], ta[:], Act.Exp,
                                 bias=0.0, scale=-0.5)

        # dense weights [65, K*OUT]
        fcwb = sb1.tile([65, KK * OUT], F32, tag="fcwb")
        if last:
            nc.sync.dma_start(out=fcwb[0:64, :], in_=ins["fc_w_l"][:])
            nc.sync.dma_start(out=fcwb[64:65, :], in_=ins["fc_b_l"][:])
        else:
            nc.sync.dma_start(out=fcwb[0:64, :], in_=ins["fc_w"][li])
            nc.sync.dma_start(out=fcwb[64:65, :], in_=ins["fc_b"][li])

        agg = sb1.tile([128, NG * OUT], F32, tag="aggsb")

        # ---- edge pipeline
        for ch in range(NCH):
            iev = sb.tile([128, g["EVN"] // 16], I16, tag="iev")
            nc.sync.dma_start(out=iev[:], in_=ins["idx_ev"][ch])
            iod = sb.tile([128, g["ODN"] // 16], I16, tag="iod")
            nc.sync.dma_start(out=iod[:], in_=ins["idx_od"][ch])
            eqt = sb.tile([128, 32 * TCH], U16, tag="eq")
            nc.sync.dma_start(out=eqt[:], in_=ins["eq"][ch])
            hg_lo = sb.tile([128, wpc * T_EV, 128], BF16, tag="hglo")
            hg_hi = sb.tile([128, wpc * T_OD, 128], BF16, tag="hghi")
            if os.environ.get("MONET_NO_GATHER", "0") == "1":
                nc.vector.memset(hg_lo[:], 0.5)
                nc.vector.memset(hg_hi[:], 0.5)
            else:
                nc.gpsimd.dma_gather(
                    out_ap=hg_lo[:], in_ap=tbl_ev, idxs_ap=iev[:],
                    num_idxs=g["EVN"], num_idxs_reg=g["EVN"],
                    elem_size=128, elem_step=256, single_packet=False)
                nc.gpsimd.dma_gather(
                    out_ap=hg_hi[:], in_ap=tbl_od, idxs_ap=iod[:],
                    num_idxs=g["ODN"], num_idxs_reg=g["ODN"],
                    elem_size=128, elem_step=256, single_packet=False)
            s3 = sb.tile([128, KK, 32, TCH], BF16, tag="s3")
            eqv = eqt[:].bitcast(BF16).rearrange("p (j t) -> p j t", t=TCH)
            for k in range(KK):
                nc.vector.tensor_tensor(
                    out=s3[:, k], in0=eqv,
                    in1=gauss[:, k, ch * TCH:(ch + 1) * TCH]
                        .rearrange("p (o t) -> p o t", o=1)
                        .broadcast_to([128, 32, TCH]),
                    op=AluOp.mult)
            for wl in range(wpc):
                win = ps.tile([65, KK * 32], F32, tag="ps")
                for tti in range(TPW):
                    tloc = wl * TPW + tti
                    if tti < T_EV:
                        lhs = hg_lo[:, wl * T_EV + tti, 0:65]
                    else:
                        lhs = hg_hi[:, wl * T_OD + (tti - T_EV), 0:65]
                    nc.tensor.matmul(out=win[:], lhsT=lhs,
                                     rhs=s3[:, :, :, tloc],
                                     start=(tti == 0), stop=(tti == TPW - 1))
                sub = wl % 4
                if sub == 0:
                    ust = sb.tile([65, KK, 4, 32], F32, tag="ust")
                nc.scalar.copy(
                    out=ust[:, :, sub, :],
                    in_=win[:].rearrange("u (k j) -> u k j", j=32))
                if sub == 3:
                    gi = (ch * wpc + wl) // 4
                    ap_ = ps.tile([128, OUT], F32, tag="ps")
                    for k in range(KK):
                        lhsu = ust[:, k].rearrange("u a b -> u (a b)")
                        nc.tensor.matmul(
                            out=ap_[:], lhsT=lhsu,
                            rhs=fcwb[:, k * OUT:(k + 1) * OUT],
                            start=(k == 0), stop=(k == KK - 1))
                    nc.scalar.copy(out=agg[:, gi * OUT:(gi + 1) * OUT], in_=ap_[:])

        # ---- BN stats (sum / sumsq over slots via ones-matmul) + AllReduce
        sq = sb1.tile([128, NG * OUT], F32, tag="sq")
        nc.scalar.square(sq[:], agg[:])
        sump = ps.tile([OUT, 1], F32, tag="ps")
        sqp = ps.tile([OUT, 1], F32, tag="ps")
        for gi in range(NG):
            nc.tensor.matmul(out=sump[:], lhsT=agg[:, gi * OUT:(gi + 1) * OUT],
                             rhs=onescol[:], start=(gi == 0), stop=(gi == NG - 1))
            nc.tensor.matmul(out=sqp[:], lhsT=sq[:, gi * OUT:(gi + 1) * OUT],
                             rhs=onescol[:], start=(gi == 0), stop=(gi == NG - 1))
        stats = sb1.tile([OUT, 2], F32, tag="stats")
        nc.scalar.copy(out=stats[:, 0:1], in_=sump[:])
        nc.scalar.copy(out=stats[:, 1:2], in_=sqp[:])
        nc.sync.dma_start(out=stats_in[0:OUT, :], in_=stats[:])
        if NO_CC:
            nc.sync.dma_start(out=stats_out[0:OUT, :], in_=stats_in[0:OUT, :])
        else:
            nc.gpsimd.collective_compute(
                "AllReduce", AluOp.add, replica_groups=[list(range(NCORES))],
                ins=[stats_in[:].opt()], outs=[stats_out[:].opt()])
        stats_ar = sb1.tile([OUT, 2], F32, tag="statsar")
        nc.sync.dma_start(out=stats_ar[:], in_=stats_out[0:OUT, :])
        trp0 = ps.tile([1, OUT], F32, tag="ps")
        nc.tensor.matmul(out=trp0[:], lhsT=stats_ar[:, 0:1],
                         rhs=ident[0:OUT, 0:OUT], start=True, stop=True)
        trp1 = ps.tile([1, OUT], F32, tag="ps")
        nc.tensor.matmul(out=trp1[:], lhsT=stats_ar[:, 1:2],
                         rhs=ident[0:OUT, 0:OUT], start=True, stop=True)
        mean = sb1.tile([1, OUT], F32, tag="mean")
        nc.vector.tensor_scalar(mean[:], trp0[:], 1.0 / nn, None, AluOp.mult)
        ev2 = sb1.tile([1, OUT], F32, tag="ev2")
        nc.vector.tensor_scalar(ev2[:], trp1[:], 1.0 / nn, None, AluOp.mult)
        m2 = sb1.tile([1, OUT], F32, tag="m2")
        nc.vector.tensor_tensor(out=m2[:], in0=mean[:], in1=mean[:], op=AluOp.mult)
        var = sb1.tile([1, OUT], F32, tag="var")
        nc.vector.tensor_tensor(out=var[:], in0=ev2[:], in1=m2[:], op=AluOp.subtract)
        nc.vector.tensor_scalar(var[:], var[:], EPS, None, AluOp.add)
        std = sb1.tile([1, OUT], F32, tag="std")
        nc.scalar.sqrt(std[:], var[:])
        rstd = sb1.tile([1, OUT], F32, tag="rstd")
        nc.vector.reciprocal(rstd[:], std[:])
        bng = sb1.tile([1, OUT], F32, tag="bng")
        bnb = sb1.tile([1, OUT], F32, tag="bnb")
        if last:
            nc.sync.dma_start(out=bng[:], in_=ins["bn_g_l"][:])
            nc.sync.dma_start(out=bnb[:], in_=ins["bn_b_l"][:])
        else:
            nc.sync.dma_start(out=bng[:], in_=ins["bn_g"][li])
            nc.sync.dma_start(out=bnb[:], in_=ins["bn_b"][li])
        sg = sb1.tile([1, OUT], F32, tag="sg")
        nc.vector.tensor_tensor(out=sg[:], in0=rstd[:], in1=bng[:], op=AluOp.mult)
        c0 = sb1.tile([1, OUT], F32, tag="c0")
        nc.vector.tensor_tensor(out=c0[:], in0=mean[:], in1=sg[:], op=AluOp.mult)
        crow = sb1.tile([1, OUT], F32, tag="crow")
        nc.vector.tensor_tensor(out=crow[:], in0=bnb[:], in1=c0[:], op=AluOp.subtract)
        reps = []
        for rsrc in (sg, crow):
            rp = ps.tile([128, OUT], F32, tag="ps")
            nc.tensor.matmul(out=rp[:], lhsT=onesrow[:], rhs=rsrc[:],
                             start=True, stop=True)
            rt = sb1.tile([128, OUT], F32, tag=f"rep{len(reps)}")
            nc.scalar.copy(out=rt[:], in_=rp[:])
            reps.append(rt)

        def rep_b(rt):
            return rt[:].rearrange("p (o c) -> p o c", o=1).broadcast_to([128, NG, OUT])

        bn = sq  # reuse buffer
        aggv = agg[:].rearrange("p (g c) -> p g c", c=OUT)
        bnv = bn[:].rearrange("p (g c) -> p g c", c=OUT)
        nc.vector.tensor_tensor(out=bnv, in0=aggv, in1=rep_b(reps[0]), op=AluOp.mult)
        nc.vector.tensor_tensor(out=bnv, in0=bnv, in1=rep_b(reps[1]), op=AluOp.add)
        nc.vector.tensor_scalar(bn[:], bn[:], 0.0, None, AluOp.max)

        if last:
            nc.sync.dma_start(out=outs["out"][:], in_=bn[:])
        else:
            h_new = sb.tile([128, NG * HID], F32, tag="h")
            nc.vector.tensor_tensor(out=h_new[:], in0=bn[:], in1=h_cur[:],
                                    op=AluOp.add)
            h_cur = h_new
            push_table(h_cur[:])

    stack.close()


# ---------------------------------------------------------------------------
# top-level entry
# ---------------------------------------------------------------------------

def _make_in_maps(g, weights):
    in_maps = []
    for c in range(NCORES):
        pc = g["per_core"][c]
        m = dict(weights)
        m["featT"] = g["featT"][c]
        m["ident"] = np.eye(g["hid"], dtype=np.float32)
        m["idx_ev"] = pc["idx_ev"]
        m["idx_od"] = pc["idx_od"]
        m["eq"] = pc["eq"]
        m["dr"] = pc["dr"]
        m["dc"] = pc["dc"]
        in_maps.append({k + "_d": v for k, v in m.items()})
    return in_maps


def _weights_dict(inputs, g):
    f32 = lambda x: np.ascontiguousarray(np.asarray(x, np.float32))
    w = dict(
        emb_w=f32(inputs["emb_w"]),                       # [128, 64]
        emb_b=f32(inputs["emb_b"]).reshape(1, -1),
        fc_w=f32(inputs["fc_w"]),                         # [3, 64, 192]
        fc_b=f32(inputs["fc_b"]).reshape(g["nhl"], 1, -1),
        mu=f32(inputs["mu"]).reshape(g["nhl"], 1, -1),
        inv_sigma=f32(inputs["inv_sigma"]).reshape(g["nhl"], 1, -1),
        pp_w=f32(inputs["pp_w"]).reshape(g["nhl"], 1, -1),
        pp_b=f32(inputs["pp_b"]).reshape(g["nhl"], 1, -1),
        bn_g=f32(inputs["bn_g"]).reshape(g["nhl"], 1, -1),
        bn_b=f32(inputs["bn_b"]).reshape(g["nhl"], 1, -1),
        fc_w_l=f32(inputs["fc_w_l"]), fc_b_l=f32(inputs["fc_b_l"]).reshape(1, -1),
        mu_l=f32(inputs["mu_l"]).reshape(1, -1),
        inv_sigma_l=f32(inputs["inv_sigma_l"]).reshape(1, -1),
        pp_w_l=f32(inputs["pp_w_l"]).reshape(1, -1),
        pp_b_l=f32(inputs["pp_b_l"]).reshape(1, -1),
        bn_g_l=f32(inputs["bn_g_l"]).reshape(1, -1),
        bn_b_l=f32(inputs["bn_b_l"]).reshape(1, -1),
    )
    return w


def _build_featT(inputs, g):
    feat = np.asarray(inputs["feature"], np.float32)
    featT = []
    for c in range(NCORES):
        arr = np.zeros((g["in_dim"], g["npc"]), np.float32)
        nds = np.flatnonzero(g["core_of"] == c)
        arr[:, g["slot_of"][nds] % g["npc"]] = feat[nds].T
        featT.append(arr)
    g["featT"] = featT


def run_device(g, weights, trace=False):
    nc = bacc.Bacc("TRN2", target_bir_lowering=False, debug=False,
                   num_devices=NCORES)
    ins_ap, outs_ap = {}, {}
    in_maps = _make_in_maps(g, weights)
    for name, arr in in_maps[0].items():
        t = nc.dram_tensor(name, list(arr.shape), mybir.dt.from_np(arr.dtype),
                           kind="ExternalInput")
        ins_ap[name[:-2]] = t.ap()
    out_t = nc.dram_tensor("out_d", [128, g["NG"] * g["ncls"]], F32,
                           kind="ExternalOutput")
    outs_ap["out"] = out_t.ap()

    with tile.TileContext(nc) as tc:
        build(tc, outs_ap, ins_ap, g)
    nc.compile()

    res = bass_utils.run_bass_kernel_spmd(
        nc, in_maps, core_ids=list(range(NCORES)), trace=trace)
    return res


def assemble_output(g, res):
    out = np.zeros((g["n"], g["ncls"]), np.float32)
    for c in range(NCORES):
        oc = res.results[c]["out_d"].reshape(128, g["NG"], g["ncls"])
        nds = np.flatnonzero(g["core_of"] == c)
        sl = g["slot_of"][nds] % g["npc"]
        out[nds] = oc[sl % 128, sl // 128, :]
    return out


def kernel(**inputs):
    g = preprocess(np.asarray(inputs["edge_index"]), GEOM_REAL)
    _build_featT(inputs, g)
    weights = _weights_dict(inputs, g)
    res = run_device(g, weights, trace=os.environ.get("MONET_TRACE", "0") == "1")
    out = assemble_output(g, res)
    kernel.last_exec_time_ns = getattr(res, "exec_time_ns", None)
    return out


# ---------------------------------------------------------------------------
# numpy reference (dev only; mirrors reference.py)
# ---------------------------------------------------------------------------

def numpy_reference(inputs, n, nhl=3):
    f = {k: np.asarray(v, np.float64 if np.asarray(v).dtype.kind == "f" else None)
         for k, v in inputs.items()}
    row, col = np.asarray(inputs["edge_index"][0]), np.asarray(inputs["edge_index"][1])
    deg_r = np.bincount(row, minlength=n)
    deg_c = np.bincount(col, minlength=n)
    srcs = 1.0 / np.sqrt(deg_r[row] + 1.0)
    dsts = 1.0 / np.sqrt(deg_c[col] + 1.0)
    pseudo = np.stack([srcs, dsts], -1)
    h = f["feature"] @ f["emb_w"] + f["emb_b"]

    def gmm(h, psd, fcw, fcb, mu, isg, bng, bnb, residual):
        kk, out = mu.shape[0], fcw.shape[1] // mu.shape[0]
        hp = (h @ fcw + fcb).reshape(n, kk, out)
        diff = psd[:, None, :] - mu
        gauss = np.exp(-0.5 * np.sum((diff * isg) ** 2, -1))
        msg = np.einsum("ek,ekc->ec", gauss, hp[row])
        agg = np.zeros((n, out))
        np.add.at(agg, col, msg)
        mean = agg.mean(0)
        var = agg.var(0)
        hbn = (agg - mean) / np.sqrt(var + EPS) * bng + bnb
        hnew = np.maximum(hbn, 0.0)
        return h + hnew if residual else hnew

    for i in range(nhl):
        psd = np.tanh(pseudo @ f["pp_w"][i] + f["pp_b"][i])
        h = gmm(h, psd, f["fc_w"][i], f["fc_b"][i], f["mu"][i],
                f["inv_sigma"][i], f["bn_g"][i], f["bn_b"][i], True)
    psd = np.tanh(pseudo @ f["pp_w_l"] + f["pp_b_l"])
    h = gmm(h, psd, f["fc_w_l"], f["fc_b_l"], f["mu_l"], f["inv_sigma_l"],
            f["bn_g_l"], f["bn_b_l"], False)
    return h.astype(np.float32)


# ---------------------------------------------------------------------------
# timed execution (repeated PJRT calls on a single compiled executable)
# ---------------------------------------------------------------------------

def run_device_timed(g, weights, n_iters=5):
    import time
    import jax
    from jax.sharding import Mesh, PartitionSpec
    from jax.experimental.shard_map import shard_map
    from concourse import bass2jax as b2j

    nc = bacc.Bacc("TRN2", target_bir_lowering=False, debug=False,
                   num_devices=NCORES)
    ins_ap = {}
    in_maps = _make_in_maps(g, weights)
    for name, arr in in_maps[0].items():
        t = nc.dram_tensor(name, list(arr.shape), mybir.dt.from_np(arr.dtype),
                           kind="ExternalInput")
        ins_ap[name[:-2]] = t.ap()
    out_t = nc.dram_tensor("out_d", [128, g["NG"] * g["ncls"]], F32,
                           kind="ExternalOutput")
    outs_ap = {"out": out_t.ap()}
    with tile.TileContext(nc) as tc:
        build(tc, outs_ap, ins_ap, g)
    nc.compile()

    b2j.install_neuronx_cc_hook()
    partition_name = (nc.partition_id_tensor.name
                      if nc.partition_id_tensor else None)
    in_names, out_names, out_avals, zero_outs = [], [], [], []
    for alloc in nc.m.functions[0].allocations:
        if not isinstance(alloc, mybir.MemoryLocationSet):
            continue
        name = alloc.memorylocations[0].name
        if alloc.kind == "ExternalInput":
            if name != partition_name:
                in_names.append(name)
        elif alloc.kind == "ExternalOutput":
            dt = mybir.dt.np(alloc.dtype)
            out_avals.append(jax.core.ShapedArray(tuple(alloc.tensor_shape), dt))
            out_names.append(name)
            zero_outs.append(np.zeros(tuple(alloc.tensor_shape), dt))
    n_params = len(in_names)
    n_outs = len(out_names)
    in_names = in_names + out_names
    if partition_name is not None:
        in_names.append(partition_name)
    donate = tuple(range(n_params, n_params + n_outs))

    def _body(*args):
        operands = list(args)
        if partition_name is not None:
            operands.append(b2j.partition_id_tensor())
        outs = b2j._bass_exec_p.bind(
            *operands,
            out_avals=tuple(out_avals),
            in_names=tuple(in_names),
            out_names=tuple(out_names),
            lowering_input_output_aliases=(),
            sim_require_finite=True,
            sim_require_nnan=True,
            nc=nc,
        )
        return tuple(outs)

    devices = jax.devices()[:NCORES]
    mesh = Mesh(np.asarray(devices), ("core",))
    sharded = jax.jit(
        shard_map(_body, mesh=mesh,
                  in_specs=(PartitionSpec("core"),) * (n_params + n_outs),
                  out_specs=(PartitionSpec("core"),) * n_outs,
                  check_rep=False),
        donate_argnums=donate, keep_unused=True)
    per_core = [[np.asarray(m[nm]) for nm in in_names[:n_params]]
                for m in in_maps]
    concat_in = [np.concatenate([per_core[c][i] for c in range(NCORES)], 0)
                 for i in range(n_params)]
    concat_in = [jax.device_put(a) for a in concat_in]

    times = []
    out_arrs = None
    for it in range(n_iters):
        czeros = [np.zeros((NCORES * z.shape[0], *z.shape[1:]), z.dtype)
                  for z in zero_outs]
        t0 = time.perf_counter()
        out_arrs = sharded(*concat_in, *czeros)
        jax.block_until_ready(out_arrs)
        times.append(time.perf_counter() - t0)
    results = [
        {nm: np.asarray(out_arrs[i]).reshape(NCORES, *out_avals[i].shape)[c]
         for i, nm in enumerate(out_names)}
        for c in range(NCORES)
    ]

    class R:
        pass
    r = R()
    r.results = results
    r.exec_time_ns = int(min(times[1:]) * 1e9) if len(times) > 1 else None
    r.all_times = times
    return r
